# revision 6
# baseline (speedup 1.0000x reference)
"""BiLSTM-CRF NLL kernel for 8 trn2 NeuronCores.

Data-parallel over batch (8 shards of 16 sequences). The whole forward
pass runs on-device per core: embedding transpose (xbar DMA), input
projections (PE, bf16), the two LSTM recurrences (PE matmuls in a
gate-chunks-on-partitions layout, ACT sigmoid/tanh, DVE cell updates),
the emissions projection, and the CRF forward algorithm rewritten in
exp-domain (plain 25x25 matmuls against exp(trans) with periodic
rescaling).  Sequence-length masking is handled by snapshotting the
scaled forward vector every step t>=255 and letting the host read off
each sequence's own final step.  The host computes the (cheap) gold-path
numerator from the returned emissions.  A pure-numpy fallback guards
against any device-path failure.
"""
import numpy as np

T, B = 512, 128
VOCAB, EMB, HID, NCLS = 32000, 256, 512, 25
H = HID // 2              # 256, per-direction hidden
PAD = 1
NCORES = 8
BS = B // NCORES          # 16 sequences per core
KAPPA = 3.2               # exp-domain growth centering: ~log(NCLS)
RESC = 8                  # rescale every RESC steps
T0SNAP = T // 2 - 1       # first snapshotted step (lengths >= T/2)


# ----------------------------------------------------------------- #
# device program
# ----------------------------------------------------------------- #
def _build(Tn):
    from contextlib import ExitStack
    import concourse.bacc as bacc
    import concourse.tile as tile
    from concourse import mybir

    F32, BF16 = mybir.dt.float32, mybir.dt.bfloat16
    AF = mybir.ActivationFunctionType
    ntok = Tn * BS            # tokens per core, t-major (tok = t*BS + b)
    NTT = ntok // 512         # 512-token tiles
    NBLK = Tn // 32           # 32-step xg/eemis blocks
    nsnap = Tn - T0SNAP if Tn > T0SNAP else 1
    t0s = T0SNAP if Tn > T0SNAP else Tn - 1

    nc = bacc.Bacc(None, target_bir_lowering=False)
    x_d = nc.dram_tensor("x", [ntok, EMB], BF16, kind="ExternalInput")
    # lhsT layouts: row = contraction index, col = dir*1024 + slot*128 + i
    wih_d = nc.dram_tensor("wih", [EMB, 2048], BF16, kind="ExternalInput")
    whh_d = nc.dram_tensor("whh", [H, 2048], BF16, kind="ExternalInput")
    bias_d = nc.dram_tensor("bias", [128, 16], F32, kind="ExternalInput")
    we_d = nc.dram_tensor("we", [HID, NCLS], BF16, kind="ExternalInput")
    # col0 = b_e ; col1 = b_e - KAPPA
    bek_d = nc.dram_tensor("bek", [NCLS, 2], F32, kind="ExternalInput")
    est_d = nc.dram_tensor("est", [NCLS, 1], F32, kind="ExternalInput")
    xmat_d = nc.dram_tensor("xmat", [NCLS, NCLS], F32, kind="ExternalInput")
    emis_d = nc.dram_tensor("emis", [NCLS, ntok], F32, kind="ExternalOutput")
    snaps_d = nc.dram_tensor("snaps", [nsnap * NCLS, BS], F32,
                             kind="ExternalOutput")
    snapm_d = nc.dram_tensor("snapm", [max(Tn // RESC, 1), BS], F32,
                             kind="ExternalOutput")

    with tile.TileContext(nc) as tc:
        with ExitStack() as ctx:
            def pool(name, bufs, space="SBUF"):
                return ctx.enter_context(
                    tc.tile_pool(name=name, bufs=bufs, space=space))

            const = pool("const", 1)
            dram = pool("dram", 1, space="DRAM")
            pj_ps = pool("pj_ps", 4, space="PSUM")
            pj_ev = pool("pj_ev", 4)
            xg_po = [pool("xgf", 2), pool("xgb", 2)]
            rc_ps = [pool("psf", 2, space="PSUM"), pool("psb", 2, space="PSUM")]
            g_po = pool("g", 4)
            sig_po = pool("sig", 4)
            gt_po = pool("gt", 4)
            u_po = pool("u", 4)
            v_po = pool("v", 4)
            c_po = [pool("cf", 3), pool("cb", 3)]
            tc_po = pool("tc", 4)
            em_ev = pool("em_ev", 2)
            ee_po = pool("ee", 2)
            s_po = pool("s", 4)
            m_po = pool("m", 2)
            r_po = pool("r", 2)

            # ---- scratch DRAM ----
            xg_dram = dram.tile([16 * 128, ntok], BF16)
            ee_dram = dram.tile([NCLS, ntok], BF16)

            # ---- constants into SBUF ----
            xT = []
            for c in range(2):
                t_ = const.tile([128, ntok], BF16, tag=f"xT{c}")
                nc.sync.dma_start_transpose(
                    out=t_[:], in_=x_d[:, c * 128:(c + 1) * 128])
                xT.append(t_)
            wih_sb, whh_sb = [], []
            for k in range(2):
                t_ = const.tile([128, 2048], BF16, tag=f"wih{k}")
                nc.sync.dma_start(out=t_[:], in_=wih_d[k * 128:(k + 1) * 128, :])
                wih_sb.append(t_)
                t2 = const.tile([128, 2048], BF16, tag=f"whh{k}")
                nc.sync.dma_start(out=t2[:], in_=whh_d[k * 128:(k + 1) * 128, :])
                whh_sb.append(t2)
            bias_sb = const.tile([128, 16], F32, tag="bias")
            nc.sync.dma_start(out=bias_sb[:], in_=bias_d[:, :])
            we_sb = []
            for k in range(4):
                t_ = const.tile([128, NCLS], BF16, tag=f"we{k}")
                nc.sync.dma_start(out=t_[:], in_=we_d[k * 128:(k + 1) * 128, :])
                we_sb.append(t_)
            bek_sb = const.tile([NCLS, 2], F32, tag="bek")
            nc.sync.dma_start(out=bek_sb[:], in_=bek_d[:, :])
            est_sb = const.tile([NCLS, 1], F32, tag="est")
            nc.sync.dma_start(out=est_sb[:], in_=est_d[:, :])
            xmat_sb = const.tile([NCLS, NCLS], F32, tag="xmat")
            nc.sync.dma_start(out=xmat_sb[:], in_=xmat_d[:, :])

            # ---- input projections: xg^T = w_ih^T.T @ x^T (+bias) ----
            def proj(d, tt):
                for half in range(2):
                    pss = []
                    for mm in range(4):
                        m = half * 4 + mm
                        ps = pj_ps.tile([128, 512], F32, tag="pj")
                        for k in range(2):
                            nc.tensor.matmul(
                                out=ps[:],
                                lhsT=wih_sb[k][:, d * 1024 + m * 128:
                                               d * 1024 + (m + 1) * 128],
                                rhs=xT[k][:, tt * 512:(tt + 1) * 512],
                                start=(k == 0), stop=(k == 1))
                        pss.append((m, ps))
                    for m, ps in pss:
                        ev = pj_ev.tile([128, 512], BF16, tag="ev")
                        nc.scalar.activation(
                            out=ev[:], in_=ps[:], func=AF.Identity,
                            bias=bias_sb[:, d * 8 + m:d * 8 + m + 1])
                        r0 = (d * 8 + m) * 128
                        nc.sync.dma_start(
                            out=xg_dram[r0:r0 + 128, tt * 512:(tt + 1) * 512],
                            in_=ev[:])

            for i in range(NTT):
                proj(0, i)
                proj(1, NTT - 1 - i)

            # ---- BiLSTM recurrence ----
            hst = [[const.tile([128, ntok], BF16, tag=f"h{d}{k}",
                               name=f"hst{d}{k}")
                    for k in range(2)] for d in range(2)]
            cprev = [None, None]

            def lstm_round(d, t, xgbuf, tl):
                first = (t == 0) if d == 0 else (t == Tn - 1)
                tprev = t - 1 if d == 0 else t + 1
                xgap = xgbuf[:, :, tl * 16:(tl + 1) * 16]
                g = g_po.tile([128, 8, 16], BF16, tag=f"g{d}")
                if first:
                    nc.vector.tensor_copy(out=g[:], in_=xgap)
                else:
                    ps = rc_ps[d].tile([128, 128], F32, tag=f"ps{d}")
                    for m in range(8):
                        for k in range(2):
                            nc.tensor.matmul(
                                out=ps[:, m * 16:(m + 1) * 16],
                                lhsT=whh_sb[k][:, d * 1024 + m * 128:
                                               d * 1024 + (m + 1) * 128],
                                rhs=hst[d][k][:, tprev * 16:(tprev + 1) * 16],
                                start=(k == 0), stop=(k == 1))
                    nc.vector.tensor_tensor(
                        out=g[:],
                        in0=ps[:].rearrange("p (s b) -> p s b", s=8),
                        in1=xgap, op=mybir.AluOpType.add)
                gf = g[:].rearrange("p s b -> p (s b)")
                sig = sig_po.tile([128, 96], BF16, tag=f"sig{d}")
                nc.scalar.activation(out=sig[:], in_=gf[:, 0:96],
                                     func=AF.Sigmoid)
                gt = gt_po.tile([128, 32], BF16, tag=f"gt{d}")
                nc.scalar.activation(out=gt[:], in_=gf[:, 96:128],
                                     func=AF.Tanh)
                u = u_po.tile([128, 32], BF16, tag=f"u{d}")
                nc.vector.tensor_mul(out=u[:], in0=sig[:, 0:32], in1=gt[:])
                if first:
                    cn = u
                else:
                    v = v_po.tile([128, 32], BF16, tag=f"v{d}")
                    nc.vector.tensor_mul(out=v[:], in0=sig[:, 32:64],
                                         in1=cprev[d][:])
                    cn = c_po[d].tile([128, 32], BF16, tag=f"c{d}")
                    nc.vector.tensor_add(out=cn[:], in0=u[:], in1=v[:])
                cprev[d] = cn
                tch = tc_po.tile([128, 32], BF16, tag=f"tc{d}")
                nc.scalar.activation(out=tch[:], in_=cn[:], func=AF.Tanh)
                for k in range(2):
                    nc.vector.tensor_mul(
                        out=hst[d][k][:, t * 16:(t + 1) * 16],
                        in0=sig[:, 64 + k * 16:64 + (k + 1) * 16],
                        in1=tch[:, k * 16:(k + 1) * 16])

            for blk in range(NBLK):
                bufs = []
                for d in range(2):
                    xb = xg_po[d].tile([128, 8, 512], BF16, tag=f"xgb{d}")
                    dblk = blk if d == 0 else NBLK - 1 - blk
                    for s in range(8):
                        r0 = (d * 8 + s) * 128
                        nc.sync.dma_start(
                            out=xb[:, s, :],
                            in_=xg_dram[r0:r0 + 128,
                                        dblk * 512:(dblk + 1) * 512])
                    bufs.append(xb)
                for tl in range(32):
                    tf = blk * 32 + tl
                    lstm_round(0, tf, bufs[0], tl)
                    tb = Tn - 1 - tf
                    lstm_round(1, tb, bufs[1], 31 - tl)

            # ---- emissions ----
            rhs_chunks = [hst[0][0], hst[0][1], hst[1][0], hst[1][1]]
            for tt in range(NTT):
                ps = pj_ps.tile([NCLS, 512], F32, tag="pj", name="em_ps_t")
                for k in range(4):
                    nc.tensor.matmul(
                        out=ps[:], lhsT=we_sb[k][:],
                        rhs=rhs_chunks[k][:, tt * 512:(tt + 1) * 512],
                        start=(k == 0), stop=(k == 3))
                ev = em_ev.tile([NCLS, 512], F32, tag="emev")
                nc.scalar.activation(out=ev[:], in_=ps[:], func=AF.Identity,
                                     bias=bek_sb[:, 0:1])
                nc.sync.dma_start(out=emis_d[:, tt * 512:(tt + 1) * 512],
                                  in_=ev[:])
                ee = em_ev.tile([NCLS, 512], BF16, tag="emee")
                nc.scalar.activation(out=ee[:], in_=ps[:], func=AF.Exp,
                                     bias=bek_sb[:, 1:2])
                nc.sync.dma_start(out=ee_dram[:, tt * 512:(tt + 1) * 512],
                                  in_=ee[:])

            # ---- CRF forward scan (exp domain) ----
            M = m_po.tile([1, BS], F32, tag="M")
            nc.vector.memset(M[:], 0.0)
            eebuf = None
            s_cur = None
            for t in range(Tn):
                if t % 32 == 0:
                    eebuf = ee_po.tile([NCLS, 512], BF16, tag="eeb")
                    nc.sync.dma_start(
                        out=eebuf[:],
                        in_=ee_dram[:, (t // 32) * 512:(t // 32 + 1) * 512])
                ees = eebuf[:, (t % 32) * 16:(t % 32 + 1) * 16]
                if t == 0:
                    s_new = s_po.tile([NCLS, BS], F32, tag="s")
                    nc.vector.tensor_scalar(
                        out=s_new[:], in0=ees, scalar1=est_sb[:, 0:1],
                        scalar2=None, op0=mybir.AluOpType.mult)
                else:
                    ps = pj_ps.tile([NCLS, BS], F32, tag="pj", name="s_ps_t")
                    nc.tensor.matmul(out=ps[:], lhsT=xmat_sb[:],
                                     rhs=s_cur[:], start=True, stop=True)
                    s_new = s_po.tile([NCLS, BS], F32, tag="s")
                    nc.vector.tensor_tensor(out=s_new[:], in0=ps[:], in1=ees,
                                            op=mybir.AluOpType.mult)
                if t % RESC == RESC - 1:
                    rf = r_po.tile([NCLS, BS], F32, tag="rf")
                    nc.vector.stream_shuffle(out=rf[:], in_=s_new[:],
                                             mask=[0] * 32)
                    rc = r_po.tile([NCLS, BS], F32, tag="rc")
                    nc.vector.reciprocal(out=rc[:], in_=rf[:])
                    lg = r_po.tile([1, BS], F32, tag="lg")
                    nc.scalar.activation(out=lg[:], in_=s_new[0:1, :],
                                         func=AF.Ln)
                    Mn = m_po.tile([1, BS], F32, tag="M")
                    nc.vector.tensor_add(out=Mn[:], in0=M[:], in1=lg[:])
                    M = Mn
                    nc.sync.dma_start(out=snapm_d[t // RESC:t // RESC + 1, :],
                                      in_=M[:])
                    s2 = s_po.tile([NCLS, BS], F32, tag="s")
                    nc.vector.tensor_mul(out=s2[:], in0=s_new[:], in1=rc[:])
                    s_new = s2
                if t >= t0s:
                    r0 = (t - t0s) * NCLS
                    nc.sync.dma_start(out=snaps_d[r0:r0 + NCLS, :],
                                      in_=s_new[:])
                s_cur = s_new

    nc.finalize()
    return nc


_NC_CACHE = {}


def _get_nc(Tn):
    if Tn not in _NC_CACHE:
        _NC_CACHE[Tn] = _build(Tn)
    return _NC_CACHE[Tn]


# ----------------------------------------------------------------- #
# host orchestration
# ----------------------------------------------------------------- #
def _perm_gates(w):
    # pytorch gate order [i,f,g,o] -> device slot order [i,f,o,g]
    return np.concatenate([w[0:2 * H // 2], w[2 * H // 2:4 * H // 2],
                           w[6 * H // 2:8 * H // 2], w[4 * H // 2:6 * H // 2]],
                          axis=0)


def _device_forward(Tn, sentence, emb, wih, whh, bias, W_e, b_e,
                    start_trans, trans):
    """Returns emis (Tn,B,NCLS) f32, snaps list, snapm list (per core)."""
    import ml_dtypes
    from concourse.bass_utils import run_bass_kernel_spmd

    bf16 = ml_dtypes.bfloat16
    nc = _get_nc(Tn)

    x_full = emb[sentence]                     # (Tn, B, EMB) f32
    wih_l = np.concatenate([_perm_gates(wih[0]).T, _perm_gates(wih[1]).T],
                           axis=1)             # (EMB, 2048)
    whh_l = np.concatenate([_perm_gates(whh[0]).T, _perm_gates(whh[1]).T],
                           axis=1)             # (H, 2048)
    bias_l = np.stack([_perm_gates(bias[0]), _perm_gates(bias[1])],
                      axis=0)                  # (2, 1024)
    bias_in = np.ascontiguousarray(
        bias_l.reshape(2, 8, 128).reshape(16, 128).T).astype(np.float32)
    we_l = np.ascontiguousarray(W_e.T).astype(bf16)          # (HID, NCLS)
    bek = np.stack([b_e, b_e - KAPPA], axis=1).astype(np.float32)
    est = np.exp(start_trans.astype(np.float64)).astype(np.float32)
    xmat = np.exp(trans.astype(np.float64)).astype(np.float32)

    wih_b = wih_l.astype(bf16)
    whh_b = whh_l.astype(bf16)
    in_maps = []
    for k in range(NCORES):
        xk = np.ascontiguousarray(
            x_full[:, k * BS:(k + 1) * BS, :].reshape(Tn * BS, EMB)
        ).astype(bf16)
        in_maps.append(dict(
            x=xk, wih=wih_b, whh=whh_b, bias=bias_in, we=we_l, bek=bek,
            est=est.reshape(NCLS, 1), xmat=xmat))
    res = run_bass_kernel_spmd(nc, in_maps, list(range(NCORES)))
    emis = np.empty((Tn, B, NCLS), np.float32)
    snaps, snapm = [], []
    for k in range(NCORES):
        r = res.results[k]
        emis[:, k * BS:(k + 1) * BS, :] = (
            r["emis"].reshape(NCLS, Tn, BS).transpose(1, 2, 0))
        snaps.append(r["snaps"])
        snapm.append(r["snapm"])
    return emis, snaps, snapm


def _numerator(emissions, tags, mask, start_trans, end_trans, trans):
    mf = mask.astype(np.float64)
    emis_at = np.take_along_axis(
        emissions.astype(np.float64), tags[..., None], axis=-1)[..., 0]
    num = start_trans[tags[0]].astype(np.float64) + emis_at[0]
    trans_sc = trans[tags[:-1], tags[1:]].astype(np.float64)
    num = num + np.sum(mf[1:] * (trans_sc + emis_at[1:]), axis=0)
    seq_ends = np.sum(mask, axis=0) - 1
    last_tags = tags[seq_ends, np.arange(tags.shape[1])]
    num = num + end_trans[last_tags].astype(np.float64)
    return num, seq_ends


def _denominator(snaps, snapm, seq_ends, end_trans):
    eend = np.exp(end_trans.astype(np.float64))
    denom = np.empty(B, np.float64)
    for k in range(NCORES):
        sn = snaps[k].astype(np.float64)      # (nsnap*NCLS, BS)
        sm = snapm[k].astype(np.float64)      # (T//RESC, BS)
        for b in range(BS):
            tstar = int(seq_ends[k * BS + b])
            svec = sn[(tstar - T0SNAP) * NCLS:(tstar - T0SNAP + 1) * NCLS, b]
            nresc = (tstar + 1) // RESC
            mval = sm[nresc - 1, b] if nresc > 0 else 0.0
            denom[k * BS + b] = (np.log(np.dot(svec, eend)) + mval
                                 + KAPPA * (tstar + 1))
    return denom


# ----------------------------------------------------------------- #
# numpy fallback (known-good baseline path)
# ----------------------------------------------------------------- #
def _sigmoid(x):
    out = np.empty_like(x)
    pos = x >= 0
    out[pos] = 1.0 / (1.0 + np.exp(-x[pos]))
    ex = np.exp(x[~pos])
    out[~pos] = ex / (1.0 + ex)
    return out


def _lstm_dir_np(x, w_ih, w_hh, b_ih, b_hh, reverse):
    Tn, Bn, _ = x.shape
    Hn = w_hh.shape[1]
    xg = x.reshape(Tn * Bn, -1) @ w_ih.T
    xg = xg.reshape(Tn, Bn, -1) + (b_ih + b_hh).astype(np.float32)
    h = np.zeros((Bn, Hn), np.float32)
    c = np.zeros((Bn, Hn), np.float32)
    hs = np.empty((Tn, Bn, Hn), np.float32)
    order = range(Tn - 1, -1, -1) if reverse else range(Tn)
    w_hhT = np.ascontiguousarray(w_hh.T)
    for t in order:
        g = xg[t] + h @ w_hhT
        i = _sigmoid(g[:, :Hn])
        f = _sigmoid(g[:, Hn:2 * Hn])
        gg = np.tanh(g[:, 2 * Hn:3 * Hn])
        o = _sigmoid(g[:, 3 * Hn:])
        c = f * c + i * gg
        h = o * np.tanh(c)
        hs[t] = h
    return hs


def _logsumexp(a, axis):
    m = np.max(a, axis=axis, keepdims=True)
    return np.log(np.sum(np.exp(a - m), axis=axis)) + np.squeeze(m, axis=axis)


def _crf_denom_np(emissions, mask, start_trans, end_trans, trans):
    score = start_trans[None, :] + emissions[0]
    for t in range(1, emissions.shape[0]):
        nxt = _logsumexp(
            score[:, :, None] + trans[None] + emissions[t][:, None, :], axis=1)
        score = np.where(mask[t][:, None], nxt, score)
    return _logsumexp(score + end_trans[None, :], axis=1)


def kernel(sentence, tags, emb,
           w_ih_f, w_hh_f, b_ih_f, b_hh_f,
           w_ih_b, w_hh_b, b_ih_b, b_hh_b,
           W_e, b_e, start_trans, end_trans, trans):
    sentence = np.asarray(sentence)
    tags = np.asarray(tags).astype(np.int64)
    f32 = lambda a: np.asarray(a, dtype=np.float32)
    emb = f32(emb)
    mask = sentence != PAD
    try:
        emis, snaps, snapm = _device_forward(
            T, sentence, emb,
            (f32(w_ih_f), f32(w_ih_b)), (f32(w_hh_f), f32(w_hh_b)),
            (f32(b_ih_f) + f32(b_hh_f), f32(b_ih_b) + f32(b_hh_b)),
            f32(W_e), f32(b_e), f32(start_trans), f32(trans))
        num, seq_ends = _numerator(emis, tags, mask, f32(start_trans),
                                   f32(end_trans), f32(trans))
        denom = _denominator(snaps, snapm, seq_ends, f32(end_trans))
        return np.float32(-np.sum(num - denom))
    except Exception:
        import traceback
        traceback.print_exc()
    # fallback: numpy
    x = emb[sentence]
    h_f = _lstm_dir_np(x, f32(w_ih_f), f32(w_hh_f), f32(b_ih_f), f32(b_hh_f),
                       reverse=False)
    h_b = _lstm_dir_np(x, f32(w_ih_b), f32(w_hh_b), f32(b_ih_b), f32(b_hh_b),
                       reverse=True)
    lstm_out = np.concatenate([h_f, h_b], axis=-1)
    emissions = (lstm_out.reshape(T * B, HID) @ f32(W_e).T
                 ).reshape(T, B, NCLS) + f32(b_e)
    num, seq_ends = _numerator(emissions, tags, mask, f32(start_trans),
                               f32(end_trans), f32(trans))
    denom = _crf_denom_np(emissions, mask, f32(start_trans), f32(end_trans),
                          f32(trans))
    return np.float32(-np.sum(num - denom))
